# revision 10
# baseline (speedup 1.0000x reference)
"""Trainium2 Bass kernel for LoraLinear:
    out = x @ W^T + 2.0 * (x @ A^T) @ B^T
    x: [4, 2048, 4096] f32, W: [4096, 4096], A: [64, 4096], B: [4096, 64]

Sharding across 8 NeuronCores: 4-way data-parallel over tokens x 2-way
tensor-parallel over out-features. Each core computes a [2048 x 2048]
output block. No collectives; the host scatters shards and gathers blocks.

Per-core device program (SPMD, same program on all 8 cores):
  - W^T shard ([4096 x 2048] fp16, 16.8 MB) is loaded once on the SP DMA
    queue and kept resident in SBUF; A^T and (2B)^T are resident too.
  - x^T streams once on the ACT DMA queue in 8 groups of 256 tokens
    (host pre-arranged so each group is one contiguous block whose
    k-slices serve both matmul shapes below).
  - Per group: xa^T = A @ x^T as 32 accumulating matmuls with a 256-wide
    moving operand; then per 128-token tile and per 512-wide out-feature
    tile, 32 base matmuls + 1 rank-64 lora matmul accumulate into one
    PSUM bank, which is copied to SBUF and stored.
  - Startup: the first group's base matmuls run k-OUTER over all 8 PSUM
    banks (2 token tiles x 4 o-tiles, ~1.75us of PE work per W block),
    consuming W^T blocks as they arrive from HBM (~1.5us/block) instead
    of stalling until the full W^T is resident.

Matmuls run in fp16 (inputs host-cast; same PE rate as bf16, 8x finer
mantissa); accumulation is fp32 in PSUM. All DMAs are simple 2D
transfers - HWDGE queue fanout for 3D shapes breaks Tile's semaphore
accounting on this stack (sim race detector confirms).
"""

import numpy as np

import concourse.mybir as mybir
import concourse.tile as tile
from concourse import bacc
from concourse.bass_utils import run_bass_kernel_spmd

# problem dims (hardcoded per harness contract)
B, S, D_IN, D_OUT, R = 4, 2048, 4096, 4096, 64
SCALING = 2.0

T_TOTAL = B * S  # 8192 tokens
DP, TP = 4, 2  # token-parallel x feature-parallel over 8 cores
T_CORE = T_TOTAL // DP  # 2048
O_CORE = D_OUT // TP  # 2048
K = D_IN  # 4096

P = 128  # SBUF partitions / matmul contraction tile
KT = K // P  # 32 k-tiles
TG_W = 2 * P  # tokens per x group (2 token tiles)
TG = T_CORE // TG_W  # 8 groups per core
NO = 512  # base matmul moving free dim (one PSUM bank of fp32)
OT = O_CORE // NO  # 4 out-feature tiles per core
X_CHUNKS = 8  # DMAs per x group (compute chases the first chunks)

MM_DT = mybir.dt.float16
MM_NP = np.float16
F32 = mybir.dt.float32

_NC_CACHE = {}


def _build_program():
    nc = bacc.Bacc()
    # xq[g][p][kt*256+u] = x^T[kt*128+p, g*256+u]  (host pre-arranged)
    xq = nc.declare_dram_parameter("xq", [TG, P, KT * TG_W], MM_DT, isOutput=False)
    wt = nc.declare_dram_parameter("wt", [K, O_CORE], MM_DT, isOutput=False)
    # ap[p][kt*64+r] = A^T[kt*128+p, r]  (host pre-arranged)
    ap = nc.declare_dram_parameter("ap", [P, KT * R], MM_DT, isOutput=False)
    bt = nc.declare_dram_parameter("bt", [R, O_CORE], MM_DT, isOutput=False)
    out = nc.declare_dram_parameter("out", [T_CORE, O_CORE], F32, isOutput=True)

    with tile.TileContext(nc) as tc:
        with (
            tc.tile_pool(name="wres", bufs=1) as wres,
            tc.tile_pool(name="xin", bufs=2) as xin,
            tc.tile_pool(name="xa", bufs=2) as xapool,
            tc.tile_pool(name="ostage", bufs=3) as ostage,
            tc.tile_pool(name="psacc", bufs=6, space="PSUM") as psacc,
            tc.tile_pool(name="psxa", bufs=2, space="PSUM") as psxa,
        ):
            # A^T and (2B)^T go first on the SP queue (~1us), then the 32
            # W^T blocks (16.8 MB, ~47us at HBM rate).
            atile = wres.tile([P, KT * R], MM_DT, name="atile")
            nc.sync.dma_start(out=atile[:], in_=ap[:])
            btile = wres.tile([R, O_CORE], MM_DT, name="btile")
            nc.sync.dma_start(out=btile[:], in_=bt[:])

            # resident W^T as 32 k-blocks side by side -> [128, 32*2048]
            wtile = wres.tile([P, KT * O_CORE], MM_DT, name="wtile")
            wt_r = wt[:].rearrange("(kt p) o -> kt p o", p=P)
            for k in range(KT):
                nc.sync.dma_start(
                    out=wtile[:, k * O_CORE : (k + 1) * O_CORE], in_=wt_r[k]
                )

            xtiles, xa_sbs = {}, {}
            chunk = KT * TG_W // X_CHUNKS

            def load_x(g):
                xt_ = xin.tile([P, KT * TG_W], MM_DT, name="xtile", tag="xtile")
                for c in range(X_CHUNKS):
                    nc.scalar.dma_start(
                        out=xt_[:, c * chunk : (c + 1) * chunk],
                        in_=xq[g][:, c * chunk : (c + 1) * chunk],
                    )
                xtiles[g] = xt_

            def compute_xa(g):
                """xa^T for 256 tokens: 32 matmuls, 256-wide moving operand."""
                ps_xa = psxa.tile([R, TG_W], F32, name="ps_xa", tag="psx")
                for k in range(KT):
                    nc.tensor.matmul(
                        ps_xa[:],
                        atile[:, k * R : (k + 1) * R],
                        xtiles[g][:, k * TG_W : (k + 1) * TG_W],
                        start=(k == 0),
                        stop=(k == KT - 1),
                    )
                xa_sb = xapool.tile([R, TG_W], MM_DT, name="xa_sb", tag="xa_sb")
                nc.vector.tensor_copy(xa_sb[:], ps_xa[:])
                xa_sbs[g] = xa_sb

            def x_slice(g, j, k):
                """lhsT for token tile j (0/1) of group g, k-block k."""
                return xtiles[g][:, k * TG_W + j * P : k * TG_W + j * P + P]

            def finish_tile(g, j, o, ps):
                """lora accumulate + copy out + store (releases the PSUM slot)."""
                nc.tensor.matmul(
                    ps[:],
                    xa_sbs[g][:, j * P : (j + 1) * P],
                    btile[:, o * NO : (o + 1) * NO],
                    start=False,
                    stop=True,
                )
                osb = ostage.tile([P, NO], F32, name="osb")
                nc.vector.tensor_copy(osb[:], ps[:])
                t = g * 2 + j
                nc.sync.dma_start(
                    out=out[t * P : (t + 1) * P, o * NO : (o + 1) * NO],
                    in_=osb[:],
                )

            def base_pass(g, j, o):
                ps = psacc.tile([P, NO], F32, name="ps", tag="ps")
                for k in range(KT):
                    nc.tensor.matmul(
                        ps[:],
                        x_slice(g, j, k),
                        wtile[:, k * O_CORE + o * NO : k * O_CORE + o * NO + NO],
                        start=(k == 0),
                        stop=False,
                    )
                finish_tile(g, j, o, ps)

            # --- startup: xa first (W-independent), then consume W blocks AS
            # THEY ARRIVE: k-outer over all 8 PSUM banks (2 token tiles x 4
            # o-tiles) so each W block gets ~1.75us of PE work vs ~1.5us
            # arrival, instead of stalling until the full W is resident.
            load_x(0)
            compute_xa(0)
            start_ps = {}
            for j in range(2):
                for o in range(OT):
                    pool, tag = (psacc, "ps") if len(start_ps) < 6 else (psxa, "psx")
                    start_ps[j, o] = pool.tile([P, NO], F32, name="ps", tag=tag)
            for k in range(KT):
                for j in range(2):
                    for o in range(OT):
                        nc.tensor.matmul(
                            start_ps[j, o][:],
                            x_slice(0, j, k),
                            wtile[:, k * O_CORE + o * NO : k * O_CORE + o * NO + NO],
                            start=(k == 0),
                            stop=False,
                        )
            for j in range(2):
                for o in range(OT):
                    finish_tile(0, j, o, start_ps[j, o])

            # --- steady state ---
            for g in range(1, TG):
                load_x(g)
                compute_xa(g)
                for j in range(2):
                    for o in range(OT):
                        base_pass(g, j, o)
    return nc


def _get_program():
    if "nc" not in _NC_CACHE:
        nc = _build_program()
        nc.finalize()  # runs Bacc.compile(): reg alloc, event-sem wait splitting
        _NC_CACHE["nc"] = nc
    return _NC_CACHE["nc"]


def _prep_x_shard(xs):
    """[T_CORE, K] f32 -> [TG, P, KT*TG_W] fp16,
    xq[g,p,kt*256+u] = xs[g*256+u, kt*128+p]."""
    x4 = xs.reshape(TG, TG_W, KT, P)  # [g, u, kt, p]
    return (
        np.ascontiguousarray(x4.transpose(0, 3, 2, 1))
        .astype(MM_NP)
        .reshape(TG, P, KT * TG_W)
    )


def _prep_in_maps(x, weight, lora_A, lora_B):
    xf = np.ascontiguousarray(x.reshape(T_TOTAL, K))

    # ap[p, kt*64+r] = A[r, kt*128+p]
    a3 = lora_A.reshape(R, KT, P)  # [r, kt, p]
    ap_host = (
        np.ascontiguousarray(a3.transpose(2, 1, 0)).astype(MM_NP).reshape(P, KT * R)
    )

    xq_shards = [_prep_x_shard(xf[d * T_CORE : (d + 1) * T_CORE]) for d in range(DP)]

    wt_shards, bt_shards = [], []
    for tp in range(TP):
        ws = weight[tp * O_CORE : (tp + 1) * O_CORE]
        wt_shards.append(np.ascontiguousarray(ws.T).astype(MM_NP))
        bs = (SCALING * lora_B[tp * O_CORE : (tp + 1) * O_CORE]).astype(np.float32)
        bt_shards.append(np.ascontiguousarray(bs.T).astype(MM_NP))

    in_maps = []
    for core in range(8):
        d, tp = core // TP, core % TP
        in_maps.append(
            {
                "xq": xq_shards[d],
                "wt": wt_shards[tp],
                "ap": ap_host,
                "bt": bt_shards[tp],
            }
        )
    return in_maps


def _gather(results):
    out = np.empty((T_TOTAL, D_OUT), dtype=np.float32)
    for core in range(8):
        d, tp = core // TP, core % TP
        out[d * T_CORE : (d + 1) * T_CORE, tp * O_CORE : (tp + 1) * O_CORE] = results[
            core
        ]["out"]
    return out.reshape(B, S, D_OUT)


def run(x, weight, lora_A, lora_B, trace=False):
    """Returns (output, BassKernelResults)."""
    nc = _get_program()
    in_maps = _prep_in_maps(
        np.asarray(x, dtype=np.float32),
        np.asarray(weight, dtype=np.float32),
        np.asarray(lora_A, dtype=np.float32),
        np.asarray(lora_B, dtype=np.float32),
    )
    res = run_bass_kernel_spmd(nc, in_maps, list(range(8)), trace=trace)
    return _gather(res.results), res


def kernel(x, weight, lora_A, lora_B):
    out, _ = run(x, weight, lora_A, lora_B, trace=False)
    return out


# revision 11
# speedup vs baseline: 1.1134x; 1.1134x over previous
"""Trainium2 Bass kernel for LoraLinear:
    out = x @ W^T + 2.0 * (x @ A^T) @ B^T
    x: [4, 2048, 4096] f32, W: [4096, 4096], A: [64, 4096], B: [4096, 64]

The LoRA update is folded into the weight on the host (merged-LoRA
inference): out = x @ (W + 2*B@A)^T, exactly. The device then runs a pure
[8192 x 4096] @ [4096 x 4096] GEMM.

Sharding across 8 NeuronCores: 4-way data-parallel over tokens x 2-way
tensor-parallel over out-features. Each core computes a [2048 x 2048]
output block. No collectives; the host scatters shards and gathers blocks.

Per-core device program (SPMD, same program on all 8 cores):
  - The merged W'^T shard ([4096 x 2048] fp16, 16.8 MB) loads once on the
    SP DMA queue and stays resident in SBUF.
  - x^T streams once on the ACT DMA queue in 8 groups of 256 tokens, each
    group as 8 chunked DMAs aligned with k-blocks so compute can chase
    the transfers.
  - Per 128-token tile and 512-wide out-feature tile: 32 accumulating
    matmuls into one PSUM bank, DVE copy to SBUF, store on the SP queue.
  - Startup: the first group's matmuls run k-OUTER across all 8 PSUM
    banks (2 token tiles x 4 o-tiles = ~1.75us of PE work per W block),
    consuming W'^T blocks as they arrive from HBM (~1.5us/block) instead
    of stalling until the full weight is resident.

Matmuls run in fp16 (inputs host-cast; same PE rate as bf16, 8x finer
mantissa); accumulation is fp32 in PSUM. All DMAs are simple 2D
transfers - HWDGE queue fanout for 3D shapes breaks Tile's semaphore
accounting on this stack (sim race detector confirms).
"""

import numpy as np

import concourse.mybir as mybir
import concourse.tile as tile
from concourse import bacc
from concourse.bass_utils import run_bass_kernel_spmd

# problem dims (hardcoded per harness contract)
B, S, D_IN, D_OUT, R = 4, 2048, 4096, 4096, 64
SCALING = 2.0

T_TOTAL = B * S  # 8192 tokens
DP, TP = 4, 2  # token-parallel x feature-parallel over 8 cores
T_CORE = T_TOTAL // DP  # 2048
O_CORE = D_OUT // TP  # 2048
K = D_IN  # 4096

P = 128  # SBUF partitions / matmul contraction tile
KT = K // P  # 32 k-tiles
TG_W = 2 * P  # tokens per x group (2 token tiles)
TG = T_CORE // TG_W  # 8 groups per core
NO = 512  # matmul moving free dim (one PSUM bank of fp32)
OT = O_CORE // NO  # 4 out-feature tiles per core
X_CHUNKS = 8  # DMAs per x group, each covering 4 k-blocks

MM_DT = mybir.dt.float16
MM_NP = np.float16
F32 = mybir.dt.float32

_NC_CACHE = {}


def _build_program():
    nc = bacc.Bacc()
    # xq[g][p][kt*256+u] = x^T[kt*128+p, g*256+u]  (host pre-arranged)
    xq = nc.declare_dram_parameter("xq", [TG, P, KT * TG_W], MM_DT, isOutput=False)
    wt = nc.declare_dram_parameter("wt", [K, O_CORE], MM_DT, isOutput=False)
    out = nc.declare_dram_parameter("out", [T_CORE, O_CORE], F32, isOutput=True)

    with tile.TileContext(nc) as tc:
        with (
            tc.tile_pool(name="wres", bufs=1) as wres,
            tc.tile_pool(name="xin", bufs=3) as xin,
            tc.tile_pool(name="ostage", bufs=4) as ostage,
            tc.tile_pool(name="psacc", bufs=8, space="PSUM") as psacc,
        ):
            # resident W'^T as 32 k-blocks side by side -> [128, 32*2048]
            wtile = wres.tile([P, KT * O_CORE], MM_DT, name="wtile")
            wt_r = wt[:].rearrange("(kt p) o -> kt p o", p=P)
            for k in range(KT):
                nc.sync.dma_start(
                    out=wtile[:, k * O_CORE : (k + 1) * O_CORE], in_=wt_r[k]
                )

            xtiles = {}
            chunk = KT * TG_W // X_CHUNKS

            def load_x(g):
                xt_ = xin.tile([P, KT * TG_W], MM_DT, name="xtile", tag="xtile")
                for c in range(X_CHUNKS):
                    nc.scalar.dma_start(
                        out=xt_[:, c * chunk : (c + 1) * chunk],
                        in_=xq[g][:, c * chunk : (c + 1) * chunk],
                    )
                xtiles[g] = xt_

            def x_slice(g, j, k):
                """lhsT for token tile j (0/1) of group g, k-block k."""
                return xtiles[g][:, k * TG_W + j * P : k * TG_W + j * P + P]

            def w_slice(k, o):
                return wtile[:, k * O_CORE + o * NO : k * O_CORE + o * NO + NO]

            def finish_tile(g, j, o, ps):
                osb = ostage.tile([P, NO], F32, name="osb")
                nc.vector.tensor_copy(osb[:], ps[:])
                t = g * 2 + j
                nc.sync.dma_start(
                    out=out[t * P : (t + 1) * P, o * NO : (o + 1) * NO],
                    in_=osb[:],
                )

            def base_pass(g, j, o):
                ps = psacc.tile([P, NO], F32, name="ps", tag="ps")
                for k in range(KT):
                    nc.tensor.matmul(
                        ps[:],
                        x_slice(g, j, k),
                        w_slice(k, o),
                        start=(k == 0),
                        stop=(k == KT - 1),
                    )
                finish_tile(g, j, o, ps)

            # --- startup: consume W blocks AS THEY ARRIVE, k-outer over all
            # 8 PSUM banks so each block gets ~1.75us of PE work vs ~1.5us
            # arrival, instead of stalling until the full W is resident.
            load_x(0)
            start_ps = {
                (j, o): psacc.tile([P, NO], F32, name="ps", tag="ps")
                for j in range(2)
                for o in range(OT)
            }
            for k in range(KT):
                for j in range(2):
                    for o in range(OT):
                        nc.tensor.matmul(
                            start_ps[j, o][:],
                            x_slice(0, j, k),
                            w_slice(k, o),
                            start=(k == 0),
                            stop=(k == KT - 1),
                        )
            for j in range(2):
                for o in range(OT):
                    finish_tile(0, j, o, start_ps[j, o])

            # --- steady state ---
            for g in range(1, TG):
                load_x(g)
                for j in range(2):
                    for o in range(OT):
                        base_pass(g, j, o)
    return nc


def _get_program():
    if "nc" not in _NC_CACHE:
        nc = _build_program()
        nc.finalize()  # runs Bacc.compile(): reg alloc, event-sem wait splitting
        _NC_CACHE["nc"] = nc
    return _NC_CACHE["nc"]


def _prep_x_shard(xs):
    """[T_CORE, K] f32 -> [TG, P, KT*TG_W] fp16,
    xq[g,p,kt*256+u] = xs[g*256+u, kt*128+p]."""
    x4 = xs.reshape(TG, TG_W, KT, P)  # [g, u, kt, p]
    return (
        np.ascontiguousarray(x4.transpose(0, 3, 2, 1))
        .astype(MM_NP)
        .reshape(TG, P, KT * TG_W)
    )


def _prep_in_maps(x, weight, lora_A, lora_B):
    xf = np.ascontiguousarray(x.reshape(T_TOTAL, K))

    # merged-LoRA weight, computed in fp32 on host: W' = W + 2*B@A
    w_merged = weight + SCALING * (lora_B @ lora_A)

    xq_shards = [_prep_x_shard(xf[d * T_CORE : (d + 1) * T_CORE]) for d in range(DP)]
    wt_shards = [
        np.ascontiguousarray(w_merged[tp * O_CORE : (tp + 1) * O_CORE].T).astype(MM_NP)
        for tp in range(TP)
    ]

    in_maps = []
    for core in range(8):
        d, tp = core // TP, core % TP
        in_maps.append({"xq": xq_shards[d], "wt": wt_shards[tp]})
    return in_maps


def _gather(results):
    out = np.empty((T_TOTAL, D_OUT), dtype=np.float32)
    for core in range(8):
        d, tp = core // TP, core % TP
        out[d * T_CORE : (d + 1) * T_CORE, tp * O_CORE : (tp + 1) * O_CORE] = results[
            core
        ]["out"]
    return out.reshape(B, S, D_OUT)


def run(x, weight, lora_A, lora_B, trace=False):
    """Returns (output, BassKernelResults)."""
    nc = _get_program()
    in_maps = _prep_in_maps(
        np.asarray(x, dtype=np.float32),
        np.asarray(weight, dtype=np.float32),
        np.asarray(lora_A, dtype=np.float32),
        np.asarray(lora_B, dtype=np.float32),
    )
    res = run_bass_kernel_spmd(nc, in_maps, list(range(8)), trace=trace)
    return _gather(res.results), res


def kernel(x, weight, lora_A, lora_B):
    out, _ = run(x, weight, lora_A, lora_B, trace=False)
    return out


# revision 12
# speedup vs baseline: 1.1170x; 1.0032x over previous
"""Trainium2 Bass kernel for LoraLinear:
    out = x @ W^T + 2.0 * (x @ A^T) @ B^T
    x: [4, 2048, 4096] f32, W: [4096, 4096], A: [64, 4096], B: [4096, 64]

The LoRA update is folded into the weight on the host (merged-LoRA
inference): out = x @ (W + 2*B@A)^T, exactly. The device then runs a pure
[8192 x 4096] @ [4096 x 4096] GEMM.

Sharding across 8 NeuronCores: 4-way data-parallel over tokens x 2-way
tensor-parallel over out-features. Each core computes a [2048 x 2048]
output block. No collectives; the host scatters shards and gathers blocks.

Per-core device program (SPMD, same program on all 8 cores):
  - The merged W'^T shard ([4096 x 2048] fp16, 16.8 MB) loads once on the
    SP DMA queue and stays resident in SBUF.
  - x^T streams once on the ACT DMA queue in 8 groups of 256 tokens, each
    group as 8 chunked DMAs aligned with k-blocks so compute can chase
    the transfers.
  - Per 128-token tile and 512-wide out-feature tile: 32 accumulating
    matmuls into one PSUM bank, DVE copy to SBUF, store on the SP queue.
  - Startup: the first group's matmuls run k-OUTER across all 8 PSUM
    banks (2 token tiles x 4 o-tiles = ~1.75us of PE work per W block),
    consuming W'^T blocks as they arrive from HBM (~1.5us/block) instead
    of stalling until the full weight is resident.

Matmuls run in fp16 (inputs host-cast; same PE rate as bf16, 8x finer
mantissa); accumulation is fp32 in PSUM. All DMAs are simple 2D
transfers - HWDGE queue fanout for 3D shapes breaks Tile's semaphore
accounting on this stack (sim race detector confirms).
"""

import numpy as np

import concourse.mybir as mybir
import concourse.tile as tile
from concourse import bacc
from concourse.bass_utils import run_bass_kernel_spmd

# problem dims (hardcoded per harness contract)
B, S, D_IN, D_OUT, R = 4, 2048, 4096, 4096, 64
SCALING = 2.0

T_TOTAL = B * S  # 8192 tokens
DP, TP = 4, 2  # token-parallel x feature-parallel over 8 cores
T_CORE = T_TOTAL // DP  # 2048
O_CORE = D_OUT // TP  # 2048
K = D_IN  # 4096

P = 128  # SBUF partitions / matmul contraction tile
KT = K // P  # 32 k-tiles
TG_W = 2 * P  # tokens per x group (2 token tiles)
TG = T_CORE // TG_W  # 8 groups per core
NO = 512  # matmul moving free dim (one PSUM bank of fp32)
OT = O_CORE // NO  # 4 out-feature tiles per core
X_CHUNKS = 16  # DMAs per x group, each covering 2 k-blocks

MM_DT = mybir.dt.float16
MM_NP = np.float16
F32 = mybir.dt.float32

_NC_CACHE = {}


def _build_program():
    nc = bacc.Bacc()
    # xq[g][p][kt*256+u] = x^T[kt*128+p, g*256+u]  (host pre-arranged)
    xq = nc.declare_dram_parameter("xq", [TG, P, KT * TG_W], MM_DT, isOutput=False)
    wt = nc.declare_dram_parameter("wt", [K, O_CORE], MM_DT, isOutput=False)
    out = nc.declare_dram_parameter("out", [T_CORE, O_CORE], F32, isOutput=True)

    with tile.TileContext(nc) as tc:
        with (
            tc.tile_pool(name="wres", bufs=1) as wres,
            tc.tile_pool(name="xin", bufs=2) as xin,
            tc.tile_pool(name="ostage", bufs=4) as ostage,
            tc.tile_pool(name="psacc", bufs=8, space="PSUM") as psacc,
        ):
            # resident W'^T as 32 k-blocks side by side -> [128, 32*2048]
            wtile = wres.tile([P, KT * O_CORE], MM_DT, name="wtile")
            wt_r = wt[:].rearrange("(kt p) o -> kt p o", p=P)
            for k in range(KT):
                nc.sync.dma_start(
                    out=wtile[:, k * O_CORE : (k + 1) * O_CORE], in_=wt_r[k]
                )

            xtiles = {}
            chunk = KT * TG_W // X_CHUNKS

            def load_x(g, after=None):
                """after: instruction the first chunk DMA must wait for —
                throttles prefetch off the HBM while the W load is critical."""
                xt_ = xin.tile([P, KT * TG_W], MM_DT, name="xtile", tag="xtile")
                for c in range(X_CHUNKS):
                    dma = nc.scalar.dma_start(
                        out=xt_[:, c * chunk : (c + 1) * chunk],
                        in_=xq[g][:, c * chunk : (c + 1) * chunk],
                    )
                    if after is not None and c == 0:
                        tile.add_dep_helper(
                            dma.ins, after.ins, reason="x prefetch throttle"
                        )
                xtiles[g] = xt_

            def x_slice(g, j, k):
                """lhsT for token tile j (0/1) of group g, k-block k."""
                return xtiles[g][:, k * TG_W + j * P : k * TG_W + j * P + P]

            def w_slice(k, o):
                return wtile[:, k * O_CORE + o * NO : k * O_CORE + o * NO + NO]

            def finish_tile(g, j, o, ps):
                osb = ostage.tile([P, NO], F32, name="osb")
                nc.vector.tensor_copy(osb[:], ps[:])
                t = g * 2 + j
                nc.sync.dma_start(
                    out=out[t * P : (t + 1) * P, o * NO : (o + 1) * NO],
                    in_=osb[:],
                )

            def base_pass(g, j, o):
                ps = psacc.tile([P, NO], F32, name="ps", tag="ps")
                for k in range(KT):
                    nc.tensor.matmul(
                        ps[:],
                        x_slice(g, j, k),
                        w_slice(k, o),
                        start=(k == 0),
                        stop=(k == KT - 1),
                    )
                finish_tile(g, j, o, ps)

            # --- startup: consume W blocks AS THEY ARRIVE, k-outer over all
            # 8 PSUM banks so each block gets ~1.75us of PE work vs ~1.5us
            # arrival, instead of stalling until the full W is resident.
            load_x(0)
            start_ps = {
                (j, o): psacc.tile([P, NO], F32, name="ps", tag="ps")
                for j in range(2)
                for o in range(OT)
            }
            gate_mm = None
            for k in range(KT):
                for j in range(2):
                    for o in range(OT):
                        mm = nc.tensor.matmul(
                            start_ps[j, o][:],
                            x_slice(0, j, k),
                            w_slice(k, o),
                            start=(k == 0),
                            stop=(k == KT - 1),
                        )
                        if k == 20 and j == 0 and o == 0:
                            gate_mm = mm
            for j in range(2):
                for o in range(OT):
                    finish_tile(0, j, o, start_ps[j, o])

            # --- steady state (g1's load gated behind most of the W load) ---
            for g in range(1, TG):
                load_x(g, after=gate_mm if g == 1 else None)
                for j in range(2):
                    for o in range(OT):
                        base_pass(g, j, o)
    return nc


def _get_program():
    if "nc" not in _NC_CACHE:
        nc = _build_program()
        nc.finalize()  # runs Bacc.compile(): reg alloc, event-sem wait splitting
        _NC_CACHE["nc"] = nc
    return _NC_CACHE["nc"]


def _prep_x_shard(xs):
    """[T_CORE, K] f32 -> [TG, P, KT*TG_W] fp16,
    xq[g,p,kt*256+u] = xs[g*256+u, kt*128+p]."""
    x4 = xs.reshape(TG, TG_W, KT, P)  # [g, u, kt, p]
    return (
        np.ascontiguousarray(x4.transpose(0, 3, 2, 1))
        .astype(MM_NP)
        .reshape(TG, P, KT * TG_W)
    )


def _prep_in_maps(x, weight, lora_A, lora_B):
    xf = np.ascontiguousarray(x.reshape(T_TOTAL, K))

    # merged-LoRA weight, computed in fp32 on host: W' = W + 2*B@A
    w_merged = weight + SCALING * (lora_B @ lora_A)

    xq_shards = [_prep_x_shard(xf[d * T_CORE : (d + 1) * T_CORE]) for d in range(DP)]
    wt_shards = [
        np.ascontiguousarray(w_merged[tp * O_CORE : (tp + 1) * O_CORE].T).astype(MM_NP)
        for tp in range(TP)
    ]

    in_maps = []
    for core in range(8):
        d, tp = core // TP, core % TP
        in_maps.append({"xq": xq_shards[d], "wt": wt_shards[tp]})
    return in_maps


def _gather(results):
    out = np.empty((T_TOTAL, D_OUT), dtype=np.float32)
    for core in range(8):
        d, tp = core // TP, core % TP
        out[d * T_CORE : (d + 1) * T_CORE, tp * O_CORE : (tp + 1) * O_CORE] = results[
            core
        ]["out"]
    return out.reshape(B, S, D_OUT)


def run(x, weight, lora_A, lora_B, trace=False):
    """Returns (output, BassKernelResults)."""
    nc = _get_program()
    in_maps = _prep_in_maps(
        np.asarray(x, dtype=np.float32),
        np.asarray(weight, dtype=np.float32),
        np.asarray(lora_A, dtype=np.float32),
        np.asarray(lora_B, dtype=np.float32),
    )
    res = run_bass_kernel_spmd(nc, in_maps, list(range(8)), trace=trace)
    return _gather(res.results), res


def kernel(x, weight, lora_A, lora_B):
    out, _ = run(x, weight, lora_A, lora_B, trace=False)
    return out


# revision 14
# speedup vs baseline: 1.1331x; 1.0145x over previous
"""Trainium2 Bass kernel for LoraLinear:
    out = x @ W^T + 2.0 * (x @ A^T) @ B^T
    x: [4, 2048, 4096] f32, W: [4096, 4096], A: [64, 4096], B: [4096, 64]

The LoRA update is folded into the weight on the host (merged-LoRA
inference): out = x @ (W + 2*B@A)^T, exactly. The device then runs a pure
[8192 x 4096] @ [4096 x 4096] GEMM.

Sharding across 8 NeuronCores: 4-way data-parallel over tokens x 2-way
tensor-parallel over out-features. Each core computes a [2048 x 2048]
output block. No collectives; the host scatters shards and gathers blocks.

Per-core device program (SPMD, same program on all 8 cores):
  - The merged W'^T shard ([4096 x 2048] fp16, 16.8 MB) loads once on the
    SP DMA queue and stays resident in SBUF.
  - x^T streams once on the ACT DMA queue in 8 groups of 256 tokens, each
    group as 8 chunked DMAs aligned with k-blocks so compute can chase
    the transfers.
  - Per 128-token tile and 512-wide out-feature tile: 32 accumulating
    matmuls into one PSUM bank, DVE copy to SBUF, store on the SP queue.
  - Startup: the first group's matmuls run k-OUTER across all 8 PSUM
    banks (2 token tiles x 4 o-tiles = ~1.75us of PE work per W block),
    consuming W'^T blocks as they arrive from HBM (~1.5us/block) instead
    of stalling until the full weight is resident.

Matmuls run in fp16 (inputs host-cast; same PE rate as bf16, 8x finer
mantissa); accumulation is fp32 in PSUM. All DMAs are simple 2D
transfers - HWDGE queue fanout for 3D shapes breaks Tile's semaphore
accounting on this stack (sim race detector confirms).
"""

import numpy as np

import concourse.mybir as mybir
import concourse.tile as tile
from concourse import bacc
from concourse.bass_utils import run_bass_kernel_spmd

# problem dims (hardcoded per harness contract)
B, S, D_IN, D_OUT, R = 4, 2048, 4096, 4096, 64
SCALING = 2.0

T_TOTAL = B * S  # 8192 tokens
DP, TP = 4, 2  # token-parallel x feature-parallel over 8 cores
T_CORE = T_TOTAL // DP  # 2048
O_CORE = D_OUT // TP  # 2048
K = D_IN  # 4096

P = 128  # SBUF partitions / matmul contraction tile
KT = K // P  # 32 k-tiles
TG_W = 2 * P  # tokens per x group (2 token tiles)
TG = T_CORE // TG_W  # 8 groups per core
NO = 512  # matmul moving free dim (one PSUM bank of fp32)
OT = O_CORE // NO  # 4 out-feature tiles per core
X_CHUNKS = 16  # DMAs per x group, each covering 2 k-blocks

MM_DT = mybir.dt.float16
MM_NP = np.float16
F32 = mybir.dt.float32

_NC_CACHE = {}


def _build_program():
    nc = bacc.Bacc()
    # xq[g][p][kt*256+u] = x^T[kt*128+p, g*256+u]  (host pre-arranged)
    xq = nc.declare_dram_parameter("xq", [TG, P, KT * TG_W], MM_DT, isOutput=False)
    wt = nc.declare_dram_parameter("wt", [K, O_CORE], MM_DT, isOutput=False)
    out = nc.declare_dram_parameter("out", [T_CORE, O_CORE], F32, isOutput=True)

    with tile.TileContext(nc) as tc:
        with (
            tc.tile_pool(name="wres", bufs=1) as wres,
            tc.tile_pool(name="xin", bufs=2) as xin,
            tc.tile_pool(name="ostage", bufs=4) as ostage,
            tc.tile_pool(name="psacc", bufs=8, space="PSUM") as psacc,
        ):
            # resident W'^T as 32 k-blocks side by side -> [128, 32*2048]
            wtile = wres.tile([P, KT * O_CORE], MM_DT, name="wtile")
            wt_r = wt[:].rearrange("(kt p) o -> kt p o", p=P)
            for k in range(KT):
                nc.sync.dma_start(
                    out=wtile[:, k * O_CORE : (k + 1) * O_CORE], in_=wt_r[k]
                )

            xtiles = {}
            chunk = KT * TG_W // X_CHUNKS

            def load_x(g, after=None):
                """after: instruction the first chunk DMA waits for —
                throttles prefetch off the HBM while W is the critical stream.
                Returns the chunk DMA instructions (for post-hoc pacing)."""
                xt_ = xin.tile([P, KT * TG_W], MM_DT, name="xtile", tag="xtile")
                dmas = []
                for c in range(X_CHUNKS):
                    dma = nc.scalar.dma_start(
                        out=xt_[:, c * chunk : (c + 1) * chunk],
                        in_=xq[g][:, c * chunk : (c + 1) * chunk],
                    )
                    if after is not None and c == 0:
                        tile.add_dep_helper(
                            dma.ins, after.ins, reason="x prefetch throttle"
                        )
                    dmas.append(dma)
                xtiles[g] = xt_
                return dmas

            def x_slice(g, j, k):
                """lhsT for token tile j (0/1) of group g, k-block k."""
                return xtiles[g][:, k * TG_W + j * P : k * TG_W + j * P + P]

            def w_slice(k, o):
                return wtile[:, k * O_CORE + o * NO : k * O_CORE + o * NO + NO]

            def finish_tile(g, j, o, ps):
                osb = ostage.tile([P, NO], F32, name="osb")
                nc.vector.tensor_copy(osb[:], ps[:])
                t = g * 2 + j
                nc.sync.dma_start(
                    out=out[t * P : (t + 1) * P, o * NO : (o + 1) * NO],
                    in_=osb[:],
                )

            def base_pass(g, j, o):
                ps = psacc.tile([P, NO], F32, name="ps", tag="ps")
                for k in range(KT):
                    nc.tensor.matmul(
                        ps[:],
                        x_slice(g, j, k),
                        w_slice(k, o),
                        start=(k == 0),
                        stop=(k == KT - 1),
                    )
                finish_tile(g, j, o, ps)

            # --- startup: consume W blocks AS THEY ARRIVE, k-outer over all
            # 8 PSUM banks so each block gets ~1.75us of PE work vs ~1.5us
            # arrival, instead of stalling until the full W is resident.
            g0_dmas = load_x(0)
            start_ps = {
                (j, o): psacc.tile([P, NO], F32, name="ps", tag="ps")
                for j in range(2)
                for o in range(OT)
            }
            k_mms = {}
            for k in range(KT):
                for j in range(2):
                    for o in range(OT):
                        mm = nc.tensor.matmul(
                            start_ps[j, o][:],
                            x_slice(0, j, k),
                            w_slice(k, o),
                            start=(k == 0),
                            stop=(k == KT - 1),
                        )
                        if j == 0 and o == 0:
                            k_mms[k] = mm
            gate_mm = k_mms[26]
            # pace g0's later x chunks ~3 chunks (6 k-blocks) ahead of use so
            # the early HBM belongs to the W stream
            for c in range(4, X_CHUNKS):
                tile.add_dep_helper(
                    g0_dmas[c].ins, k_mms[2 * c - 6].ins, reason="x chunk pacing"
                )
            for j in range(2):
                for o in range(OT):
                    finish_tile(0, j, o, start_ps[j, o])

            # --- steady state (g1's load gated behind most of the W load) ---
            for g in range(1, TG):
                load_x(g, after=gate_mm if g == 1 else None)
                for j in range(2):
                    for o in range(OT):
                        base_pass(g, j, o)
    return nc


def _get_program():
    if "nc" not in _NC_CACHE:
        nc = _build_program()
        nc.finalize()  # runs Bacc.compile(): reg alloc, event-sem wait splitting
        _NC_CACHE["nc"] = nc
    return _NC_CACHE["nc"]


def _prep_x_shard(xs):
    """[T_CORE, K] f32 -> [TG, P, KT*TG_W] fp16,
    xq[g,p,kt*256+u] = xs[g*256+u, kt*128+p]."""
    x4 = xs.reshape(TG, TG_W, KT, P)  # [g, u, kt, p]
    return (
        np.ascontiguousarray(x4.transpose(0, 3, 2, 1))
        .astype(MM_NP)
        .reshape(TG, P, KT * TG_W)
    )


def _prep_in_maps(x, weight, lora_A, lora_B):
    xf = np.ascontiguousarray(x.reshape(T_TOTAL, K))

    # merged-LoRA weight, computed in fp32 on host: W' = W + 2*B@A
    w_merged = weight + SCALING * (lora_B @ lora_A)

    xq_shards = [_prep_x_shard(xf[d * T_CORE : (d + 1) * T_CORE]) for d in range(DP)]
    wt_shards = [
        np.ascontiguousarray(w_merged[tp * O_CORE : (tp + 1) * O_CORE].T).astype(MM_NP)
        for tp in range(TP)
    ]

    in_maps = []
    for core in range(8):
        d, tp = core // TP, core % TP
        in_maps.append({"xq": xq_shards[d], "wt": wt_shards[tp]})
    return in_maps


def _gather(results):
    out = np.empty((T_TOTAL, D_OUT), dtype=np.float32)
    for core in range(8):
        d, tp = core // TP, core % TP
        out[d * T_CORE : (d + 1) * T_CORE, tp * O_CORE : (tp + 1) * O_CORE] = results[
            core
        ]["out"]
    return out.reshape(B, S, D_OUT)


def run(x, weight, lora_A, lora_B, trace=False):
    """Returns (output, BassKernelResults)."""
    nc = _get_program()
    in_maps = _prep_in_maps(
        np.asarray(x, dtype=np.float32),
        np.asarray(weight, dtype=np.float32),
        np.asarray(lora_A, dtype=np.float32),
        np.asarray(lora_B, dtype=np.float32),
    )
    res = run_bass_kernel_spmd(nc, in_maps, list(range(8)), trace=trace)
    return _gather(res.results), res


def kernel(x, weight, lora_A, lora_B):
    out, _ = run(x, weight, lora_A, lora_B, trace=False)
    return out
